# revision 2
# baseline (speedup 1.0000x reference)
"""CrossAttentionPool forward on 8 TRN2 NeuronCores.

Reference computation (per batch b):
    q = lines[b] @ w_q.T ; k = videos[b] @ w_k.T
    scores = (q @ k.T) * D**-0.5, masked where video_mask==0
    out = softmax(scores, axis=-1) @ videos[b]

Strategy (data-parallel over batch, 4 batches/core):
    scores = lines @ W @ videos^T with W = (w_q.T @ w_k) * scale folded on host.
    All device tensors are bf16; ~4e-3 rel err fits the 2e-2 gate.
        u[d,(p,v)] = sum_d' W[d,d'] videosT[d',(p,v)]    (per pair p)
        scoresT[v,l] = sum_d  u[d,v] linesT[d,l]         (per l-half)
        eT = exp(scoresT + mask_bias[v])                 (ScalarE, per l-half)
        out[l,:]||sum[l] = eT[v,l-chunk]^T @ vbr_ext     (vbr has ones col 768)
        osb = out * 1/sum  (per-chunk reciprocal + scale, Scalar/Vector)

    Schedule notes (from trace analysis of the 43.2us baseline):
    - Both HWDGE rings (sync+scalar) carry ~2.95 MB each, every tensor split
      into c-halves across the rings, ordered by first use: vT01, wl m-pairs,
      vT23 (so u23 can follow u01 on PE immediately), lT0, lT1, vbr, lT2, lT3.
    - 12 warm-up matmuls (not 28): PE ramps 1.2->2.4GHz ~5.2us after first
      activity; warm ends right as vT01+wl m01 land so u01 is never queued
      behind dead work.
    - scores are computed per l-half (two 256-col accum groups per psT bank)
      so exp of half a overlaps half b's matmuls; out chunks 0,1 only need
      exp half a.
    - The softmax denominator rides the out matmul as moving-operand col 768
      (ones baked into vbr on host): no sums matmuls, one [128,1] reciprocal
      per chunk on Vector feeding the per-partition scale.
    - Stores: half-batch [128,2,768] on the sync ring (scalar ring keeps
      loading; scalar ENGINE keeps exp/scale cadence); last batch per-chunk
      on alternating rings so the final transfer is small and early.
"""
import numpy as np
import concourse.bacc as bacc
import concourse.tile as tile
from concourse import mybir
from concourse.bass_utils import run_bass_kernel_spmd

N_CORES = 8
B, L, V, D = 32, 512, 128, 768
BPC = B // N_CORES          # batches per core
KC = D // 128               # 6 contraction chunks
LC = L // 128               # 4 line chunks
DE = D + 1                  # vbr width incl. ones column
F32 = mybir.dt.float32
BF16 = mybir.dt.bfloat16
WARM_N = 12


def _body(tc, out_d, linesT_d, vT01_d, vT23_d, vones_d, maskb_d, wl_d):
    nc = tc.nc
    from contextlib import ExitStack
    with ExitStack() as ctx:
        const = ctx.enter_context(tc.tile_pool(name="const", bufs=1))
        persist = ctx.enter_context(tc.tile_pool(name="persist", bufs=1))
        etpool = ctx.enter_context(tc.tile_pool(name="etp", bufs=4))
        outpool = ctx.enter_context(tc.tile_pool(name="osb", bufs=6))
        rpool = ctx.enter_context(tc.tile_pool(name="rp", bufs=8))

        # PSUM: 8 banks.  pp_st (2 bufs x 1 bank): score tiles [128,512] f32,
        # each holding two 256-col accumulation groups (l-halves); the warm-up
        # target rides this slot set.  pp_o (3 bufs x 2 banks): u accumulators
        # and out tiles [128,1024].
        pp_st = ctx.enter_context(tc.tile_pool(name="pp_st", bufs=2, space="PSUM"))
        pp_o = ctx.enter_context(tc.tile_pool(name="pp_o", bufs=3, space="PSUM"))

        maskb = const.tile([128, BPC], F32)
        # tiny per-partition rows -> SWDGE path, never blocks a ring head.
        nc.gpsimd.dma_start(maskb[:], maskb_d[:])

        # persistent input tiles
        wl_r = persist.tile([128, KC, KC, 128], BF16, tag="wlr")
        wl_v = wl_d[:].rearrange("p (m c s) -> p m c s", m=KC, c=KC)
        vT01 = persist.tile([128, KC, 2 * V], BF16, tag="vT01")
        vT23 = persist.tile([128, KC, 2 * V], BF16, tag="vT23")
        lT = [persist.tile([128, KC, L], BF16, tag=f"lT{b}", name=f"lT{b}")
              for b in range(BPC)]
        vbr = persist.tile([128, BPC, DE], BF16, tag="vbr")

        vT01_v = vT01_d[:].rearrange("p (c w) -> p c w", w=2 * V)
        vT23_v = vT23_d[:].rearrange("p (c w) -> p c w", w=2 * V)
        lT_v = [linesT_d[b].rearrange("p (c w) -> p c w", w=L)
                for b in range(BPC)]
        vbr_v = vones_d[:].rearrange("p (b w) -> p b w", w=DE)

        # Balanced load ladder: ~2.95MB per ring, ordered by first use.
        H = KC // 2
        nc.sync.dma_start(vT01[:, 0:H], vT01_v[:, 0:H])
        nc.scalar.dma_start(vT01[:, H:KC], vT01_v[:, H:KC])
        nc.sync.dma_start(wl_r[:, 0:1], wl_v[:, 0:1])
        nc.scalar.dma_start(wl_r[:, 1:2], wl_v[:, 1:2])
        nc.sync.dma_start(wl_r[:, 2:3], wl_v[:, 2:3])
        nc.scalar.dma_start(wl_r[:, 3:4], wl_v[:, 3:4])
        nc.sync.dma_start(wl_r[:, 4:5], wl_v[:, 4:5])
        nc.scalar.dma_start(wl_r[:, 5:6], wl_v[:, 5:6])
        nc.sync.dma_start(vT23[:, 0:H], vT23_v[:, 0:H])
        nc.scalar.dma_start(vT23[:, H:KC], vT23_v[:, H:KC])
        nc.sync.dma_start(lT[0][:, 0:H], lT_v[0][:, 0:H])
        nc.scalar.dma_start(lT[0][:, H:KC], lT_v[0][:, H:KC])
        nc.sync.dma_start(lT[1][:, 0:H], lT_v[1][:, 0:H])
        nc.scalar.dma_start(lT[1][:, H:KC], lT_v[1][:, H:KC])
        nc.sync.dma_start(vbr[:, 0:2], vbr_v[:, 0:2])
        nc.scalar.dma_start(vbr[:, 2:4], vbr_v[:, 2:4])
        nc.sync.dma_start(lT[2][:, 0:H], lT_v[2][:, 0:H])
        nc.scalar.dma_start(lT[2][:, H:KC], lT_v[2][:, H:KC])
        nc.sync.dma_start(lT[3][:, 0:H], lT_v[3][:, 0:H])
        nc.scalar.dma_start(lT[3][:, H:KC], lT_v[3][:, H:KC])

        # u pair tiles: [128, (m, 2, v)] (partition = d within chunk m)
        u01 = persist.tile([128, KC, 2 * V], BF16, tag="u01")
        u23 = persist.tile([128, KC, 2 * V], BF16, tag="u23")
        us = {0: u01, 1: u23}
        vs = {0: vT01, 1: vT23}
        eTs = {}

        def u_mm2(pair, m):
            # two m-chunks share one accumulator slot -> one PSUM->SBUF copy
            pu = pp_o.tile([128, 1024], F32, name="po")
            for mm in (m, m + 1):
                off = (mm - m) * 256
                for c in range(KC):
                    nc.tensor.matmul(pu[:, off:off + 256],
                                     wl_r[:, mm, c], vs[pair][:, c],
                                     start=(c == 0), stop=(c == KC - 1))
            nc.vector.tensor_copy(us[pair][:, m:m + 2], pu[:, 0:512])

        def score_b(b):
            # scores in two l-halves; exp of half h fires right after its
            # 6-matmul chain so out chunks 2h,2h+1 unblock early.
            psT = pp_st.tile([128, L], F32, name="psT")
            eTs[b] = etpool.tile([128, L], BF16, name="eT")
            for h in (0, 1):
                sl = slice(h * 256, (h + 1) * 256)
                for m in range(KC):
                    nc.tensor.matmul(psT[:, sl],
                                     us[b // 2][:, m, (b % 2) * V:(b % 2 + 1) * V],
                                     lT[b][:, m, sl],
                                     start=(m == 0), stop=(m == KC - 1))
                nc.scalar.activation(eTs[b][:, sl], psT[:, sl],
                                     mybir.ActivationFunctionType.Exp,
                                     bias=maskb[:, b:b + 1])

        osbs = {}

        def out_chunk(b, i, scale_eng):
            eT = eTs[b]
            po = pp_o.tile([128, 1024], F32, name="po")
            nc.tensor.matmul(po[:, 0:512], eT[:, i * 128:(i + 1) * 128],
                             vbr[:, b, 0:512], start=True, stop=True)
            # cols 512:769 -- col 768 (ones) gives the softmax denominator
            nc.tensor.matmul(po[:, 512:512 + 257], eT[:, i * 128:(i + 1) * 128],
                             vbr[:, b, 512:DE], start=True, stop=True)
            rec = rpool.tile([128, 1], F32, name="rec")
            nc.vector.reciprocal(rec[:], po[:, 768:769])
            if i % 2 == 0:
                osbs[b] = outpool.tile([128, 2, D], BF16, name="osb")
            osb = osbs[b]
            if scale_eng == "split":
                nc.scalar.mul(osb[:, i % 2, 0:384], po[:, 0:384], rec[:])
                nc.vector.tensor_scalar_mul(osb[:, i % 2, 384:768],
                                            po[:, 384:768], rec[:])
            elif scale_eng == "scalar":
                nc.scalar.mul(osb[:, i % 2], po[:, 0:768], rec[:])
            else:
                nc.vector.tensor_scalar_mul(osb[:, i % 2], po[:, 0:768], rec[:])
            dst = out_d[b].rearrange("(i p) d -> p i d", p=128)
            if b == BPC - 1:
                # last batch: store each l-chunk alone, alternating rings
                oeng = nc.sync if i % 2 == 0 else nc.scalar
                oeng.dma_start(dst[:, i:i + 1], osb[:, i % 2:i % 2 + 1])
            elif i % 2 == 1:
                # store per half-batch on the sync ring (in-order behind the
                # remaining loads, so it never delays them)
                nc.sync.dma_start(dst[:, i - 1:i + 1], osb[:])

        # ---- PE program ----
        # Warm-up: trigger the DVFS ramp; sized to end as the first input
        # slices land (~10us), so real matmuls are never queued behind it.
        warm = const.tile([128, 256], BF16)
        nc.vector.memset(warm[:], 0.0)
        pw = pp_st.tile([128, L], F32, name="psT")
        for _ in range(WARM_N):
            nc.tensor.matmul(pw[:, 0:256], warm[:, 0:128], warm[:],
                             start=True, stop=True)

        u_mm2(0, 0)
        u_mm2(0, 2)
        u_mm2(0, 4)
        score_b(0)
        u_mm2(1, 0)
        out_chunk(0, 0, "scalar")
        out_chunk(0, 1, "vector")
        u_mm2(1, 2)
        out_chunk(0, 2, "scalar")
        out_chunk(0, 3, "vector")
        u_mm2(1, 4)
        score_b(1)
        out_chunk(1, 0, "scalar")
        out_chunk(1, 1, "vector")
        out_chunk(1, 2, "scalar")
        out_chunk(1, 3, "vector")
        score_b(2)
        out_chunk(2, 0, "scalar")
        out_chunk(2, 1, "vector")
        out_chunk(2, 2, "scalar")
        out_chunk(2, 3, "vector")
        score_b(3)
        out_chunk(3, 0, "vector")
        out_chunk(3, 1, "scalar")
        out_chunk(3, 2, "vector")
        out_chunk(3, 3, "split")


_CACHE = {}


def _build():
    if "nc" in _CACHE:
        return _CACHE["nc"]
    nc = bacc.Bacc("TRN2", target_bir_lowering=False, debug=False,
                   num_devices=N_CORES)
    linesT_d = nc.dram_tensor("linesT", [BPC, 128, KC * L], BF16,
                              kind="ExternalInput").ap()
    vT01_d = nc.dram_tensor("vT01", [128, KC * 2 * V], BF16,
                            kind="ExternalInput").ap()
    vT23_d = nc.dram_tensor("vT23", [128, KC * 2 * V], BF16,
                            kind="ExternalInput").ap()
    vones_d = nc.dram_tensor("vones", [128, BPC * DE], BF16,
                             kind="ExternalInput").ap()
    maskb_d = nc.dram_tensor("maskb", [V, BPC], F32, kind="ExternalInput").ap()
    wl_d = nc.dram_tensor("wl", [128, KC * D], BF16, kind="ExternalInput").ap()
    out_d = nc.dram_tensor("out", [BPC, L, D], BF16, kind="ExternalOutput").ap()
    with tile.TileContext(nc) as tc:
        _body(tc, out_d, linesT_d, vT01_d, vT23_d, vones_d, maskb_d, wl_d)
    nc.compile()
    _CACHE["nc"] = nc
    return nc


def _in_maps(lines, videos, video_mask, w_q, w_k):
    w_q = np.asarray(w_q, dtype=np.float32)
    w_k = np.asarray(w_k, dtype=np.float32)
    video_mask = np.asarray(video_mask)
    scale = np.float64(D) ** -0.5
    # scores = lines @ (w_q.T @ w_k * scale) @ videos^T; device wants WL[d', d] = W[d, d']
    WL = (scale * (w_k.astype(np.float64).T @ w_q.astype(np.float64))
          ).astype(np.float32)
    mask_bias = np.where(np.asarray(video_mask) == 0,
                         np.float32(-50.0), np.float32(0.0)).astype(np.float32)
    import ml_dtypes
    bf16 = ml_dtypes.bfloat16
    videos = np.asarray(videos, dtype=np.float32)
    lines = np.asarray(lines, dtype=np.float32)
    # vbr layout [v, (b, d)] per core, with a ones column appended (d=768)
    vones = videos.astype(bf16)
    vones = vones.reshape(N_CORES, BPC, V, D).transpose(0, 2, 1, 3)
    ones_col = np.ones((N_CORES, V, BPC, 1), dtype=bf16)
    vones = np.concatenate([vones, ones_col], axis=-1)
    vones = np.ascontiguousarray(vones.reshape(N_CORES, V, BPC * DE))
    # lT layout [b][p=d%128, (c=d//128, l)] per core
    linesT = lines.transpose(0, 2, 1).astype(bf16)          # [B, D, L]
    linesT = linesT.reshape(B, KC, 128, L).transpose(0, 2, 1, 3)
    linesT = np.ascontiguousarray(linesT.reshape(N_CORES, BPC, 128, KC * L))
    # vT pair layout [p=d'%128, (c, bpair, v)] per core
    videosT = videos.transpose(0, 2, 1).astype(bf16)        # [B, D, V]
    videosT = videosT.reshape(N_CORES, BPC, KC, 128, V).transpose(0, 3, 2, 1, 4)
    vT01 = np.ascontiguousarray(
        videosT[:, :, :, 0:2, :].reshape(N_CORES, 128, KC * 2 * V))
    vT23 = np.ascontiguousarray(
        videosT[:, :, :, 2:4, :].reshape(N_CORES, 128, KC * 2 * V))
    # wl layout [p=d'%128, (m, c, s)] with wl[p, m, c, s] = WL[c*128+p, m*128+s]
    WLh = np.ascontiguousarray(
        WL.astype(bf16).reshape(KC, 128, KC, 128)
        .transpose(1, 2, 0, 3).reshape(128, KC * D))
    maps = []
    for c in range(N_CORES):
        sl = slice(c * BPC, (c + 1) * BPC)
        maps.append({
            "linesT": linesT[c],
            "vT01": vT01[c],
            "vT23": vT23[c],
            "vones": vones[c],
            "maskb": np.ascontiguousarray(mask_bias[sl].T),
            "wl": WLh,
        })
    return maps


def kernel(lines, videos, video_mask, w_q, w_k):
    nc = _build()
    maps = _in_maps(lines, videos, video_mask, w_q, w_k)
    res = run_bass_kernel_spmd(nc, maps, list(range(N_CORES)))
    out = np.concatenate([res.results[c]["out"] for c in range(N_CORES)], axis=0)
    return np.ascontiguousarray(out.astype(np.float32))


# revision 6
# speedup vs baseline: 1.0213x; 1.0213x over previous
"""CrossAttentionPool forward on 8 TRN2 NeuronCores.

Reference computation (per batch b):
    q = lines[b] @ w_q.T ; k = videos[b] @ w_k.T
    scores = (q @ k.T) * D**-0.5, masked where video_mask==0
    out = softmax(scores, axis=-1) @ videos[b]

Strategy (data-parallel over batch, 4 batches/core):
    scores = lines @ W @ videos^T with W = (w_q.T @ w_k) * scale folded on host.
    All device tensors are bf16; ~4e-3 rel err fits the 2e-2 gate.
        u[d,(p,v)] = sum_d' W[d,d'] videosT[d',(p,v)]    (per pair p)
        scoresT[v,l] = sum_d  u[d,v] linesT[d,l]         (per l-half)
        eT = exp(scoresT + mask_bias[v])                 (ScalarE, per l-half)
        out[l,:]||sum[l] = eT[v,l-chunk]^T @ vbr_ext     (vbr has ones col 768)
        osb = out * 1/sum  (per-chunk reciprocal + scale, Scalar/Vector)

    Schedule notes (from trace analysis of the 43.2us baseline):
    - Both HWDGE rings (sync+scalar) carry ~2.95 MB each, every tensor split
      into c-halves across the rings, ordered by first use: vT01, wl m-pairs,
      vT23 (so u23 can follow u01 on PE immediately), lT0, lT1, vbr, lT2, lT3.
    - 12 warm-up matmuls (not 28): PE ramps 1.2->2.4GHz ~5.2us after first
      activity; warm ends right as vT01+wl m01 land so u01 is never queued
      behind dead work.
    - scores are computed per l-half (two 256-col accum groups per psT bank)
      so exp of half a overlaps half b's matmuls; out chunks 0,1 only need
      exp half a.
    - The softmax denominator rides the out matmul as moving-operand col 768
      (ones baked into vbr on host): no sums matmuls, one [128,1] reciprocal
      per chunk on Vector feeding the per-partition scale.
    - Stores: half-batch [128,2,768] on the sync ring (scalar ring keeps
      loading; scalar ENGINE keeps exp/scale cadence); last batch per-chunk
      on alternating rings so the final transfer is small and early.
"""
import numpy as np
import concourse.bacc as bacc
import concourse.tile as tile
from concourse import mybir
from concourse.bass_utils import run_bass_kernel_spmd

N_CORES = 8
B, L, V, D = 32, 512, 128, 768
BPC = B // N_CORES          # batches per core
KC = D // 128               # 6 contraction chunks
LC = L // 128               # 4 line chunks
DE = D + 1                  # vbr width incl. ones column
F32 = mybir.dt.float32
BF16 = mybir.dt.bfloat16
WARM_N = 13


def _body(tc, out_d, linesT_d, vT01_d, vT23_d, vones_d, maskb_d, wl_d):
    nc = tc.nc
    from contextlib import ExitStack
    with ExitStack() as ctx:
        const = ctx.enter_context(tc.tile_pool(name="const", bufs=1))
        persist = ctx.enter_context(tc.tile_pool(name="persist", bufs=1))
        etpool = ctx.enter_context(tc.tile_pool(name="etp", bufs=4))
        outpool = ctx.enter_context(tc.tile_pool(name="osb", bufs=6))
        rpool = ctx.enter_context(tc.tile_pool(name="rp", bufs=8))

        # PSUM: 8 banks.  pp_st (2 bufs x 1 bank): score tiles [128,512] f32,
        # each holding two 256-col accumulation groups (l-halves); the warm-up
        # target rides this slot set.  pp_o (3 bufs x 2 banks): u accumulators
        # and out tiles [128,1024].
        pp_st = ctx.enter_context(tc.tile_pool(name="pp_st", bufs=2, space="PSUM"))
        pp_o = ctx.enter_context(tc.tile_pool(name="pp_o", bufs=3, space="PSUM"))

        maskb = const.tile([128, BPC], F32)
        # tiny per-partition rows -> SWDGE path, never blocks a ring head.
        nc.gpsimd.dma_start(maskb[:], maskb_d[:])

        # persistent input tiles
        wl_r = persist.tile([128, KC, KC, 128], BF16, tag="wlr")
        wl_v = wl_d[:].rearrange("p (m c s) -> p m c s", m=KC, c=KC)
        vT01 = persist.tile([128, KC, 2 * V], BF16, tag="vT01")
        vT23 = persist.tile([128, KC, 2 * V], BF16, tag="vT23")
        lT = [persist.tile([128, KC, L], BF16, tag=f"lT{b}", name=f"lT{b}")
              for b in range(BPC)]
        vbr = persist.tile([128, BPC, DE], BF16, tag="vbr")

        vT01_v = vT01_d[:].rearrange("p (c w) -> p c w", w=2 * V)
        vT23_v = vT23_d[:].rearrange("p (c w) -> p c w", w=2 * V)
        lT_v = [linesT_d[b].rearrange("p (c w) -> p c w", w=L)
                for b in range(BPC)]
        vbr_v = vones_d[:].rearrange("p (b w) -> p b w", w=DE)

        # Balanced load ladder, ordered by first use.  Every dispatch keeps
        # >=3072B per-partition elements (1536B packets cost ~15% ring rate):
        # wl moves in m-pairs, vT01/vT23 whole, lT/vbr in c-halves.
        H = KC // 2
        nc.sync.dma_start(vT01[:], vT01_v[:])
        nc.scalar.dma_start(wl_r[:, 2:4], wl_v[:, 2:4])
        nc.sync.dma_start(wl_r[:, 0:2], wl_v[:, 0:2])
        nc.scalar.dma_start(wl_r[:, 4:6], wl_v[:, 4:6])
        nc.sync.dma_start(lT[0][:, 0:H], lT_v[0][:, 0:H])
        nc.scalar.dma_start(lT[0][:, H:KC], lT_v[0][:, H:KC])
        nc.sync.dma_start(vbr[:, 0:2], vbr_v[:, 0:2])
        nc.scalar.dma_start(vT23[:], vT23_v[:])
        nc.sync.dma_start(lT[1][:, 0:H], lT_v[1][:, 0:H])
        nc.scalar.dma_start(lT[1][:, H:KC], lT_v[1][:, H:KC])
        nc.sync.dma_start(lT[2][:, 0:H], lT_v[2][:, 0:H])
        nc.scalar.dma_start(lT[2][:, H:KC], lT_v[2][:, H:KC])
        nc.sync.dma_start(lT[3][:, 0:H], lT_v[3][:, 0:H])
        nc.scalar.dma_start(lT[3][:, H:KC], lT_v[3][:, H:KC])
        nc.scalar.dma_start(vbr[:, 2:4], vbr_v[:, 2:4])

        # u pair tiles: [128, (m, 2, v)] (partition = d within chunk m)
        u01 = persist.tile([128, KC, 2 * V], BF16, tag="u01")
        u23 = persist.tile([128, KC, 2 * V], BF16, tag="u23")
        us = {0: u01, 1: u23}
        vs = {0: vT01, 1: vT23}
        eTs = {}

        def u_mm2(pair, m):
            # two m-chunks share one accumulator slot -> one PSUM->SBUF copy
            pu = pp_o.tile([128, 1024], F32, name="po")
            for mm in (m, m + 1):
                off = (mm - m) * 256
                for c in range(KC):
                    nc.tensor.matmul(pu[:, off:off + 256],
                                     wl_r[:, mm, c], vs[pair][:, c],
                                     start=(c == 0), stop=(c == KC - 1))
            nc.vector.tensor_copy(us[pair][:, m:m + 2], pu[:, 0:512])

        def score_b(b):
            # scores in two l-halves; exp of half h fires right after its
            # 6-matmul chain so out chunks 2h,2h+1 unblock early.
            psT = pp_st.tile([128, L], F32, name="psT")
            eTs[b] = etpool.tile([128, L], BF16, name="eT")
            for h in (0, 1):
                sl = slice(h * 256, (h + 1) * 256)
                for m in range(KC):
                    nc.tensor.matmul(psT[:, sl],
                                     us[b // 2][:, m, (b % 2) * V:(b % 2 + 1) * V],
                                     lT[b][:, m, sl],
                                     start=(m == 0), stop=(m == KC - 1))
                nc.scalar.activation(eTs[b][:, sl], psT[:, sl],
                                     mybir.ActivationFunctionType.Exp,
                                     bias=maskb[:, b:b + 1])

        osbs = {}

        def out_chunk(b, i, scale_eng):
            eT = eTs[b]
            po = pp_o.tile([128, 1024], F32, name="po")
            nc.tensor.matmul(po[:, 0:512], eT[:, i * 128:(i + 1) * 128],
                             vbr[:, b, 0:512], start=True, stop=True)
            # cols 512:769 -- col 768 (ones) gives the softmax denominator
            nc.tensor.matmul(po[:, 512:512 + 257], eT[:, i * 128:(i + 1) * 128],
                             vbr[:, b, 512:DE], start=True, stop=True)
            rec = rpool.tile([128, 1], F32, name="rec")
            nc.vector.reciprocal(rec[:], po[:, 768:769])
            if i % 2 == 0:
                osbs[b] = outpool.tile([128, 2, D], BF16, name="osb")
            osb = osbs[b]
            if scale_eng == "split":
                nc.scalar.mul(osb[:, i % 2, 0:384], po[:, 0:384], rec[:])
                nc.vector.tensor_scalar_mul(osb[:, i % 2, 384:768],
                                            po[:, 384:768], rec[:])
            elif scale_eng == "scalar":
                nc.scalar.mul(osb[:, i % 2], po[:, 0:768], rec[:])
            else:
                nc.vector.tensor_scalar_mul(osb[:, i % 2], po[:, 0:768], rec[:])
            dst = out_d[b].rearrange("(i p) d -> p i d", p=128)
            if b == BPC - 1:
                # last batch: store each l-chunk alone, alternating rings
                # (scalar engine is free by now; small final transfers)
                oeng = nc.sync if i % 2 == 0 else nc.scalar
                oeng.dma_start(dst[:, i:i + 1], osb[:, i % 2:i % 2 + 1])
            elif i % 2 == 1:
                # store per half-batch, dispatched by the SYNC engine only:
                # scalar engine must keep its exp/scale cadence, and ring
                # FIFO puts these behind the remaining loads harmlessly.
                nc.sync.dma_start(dst[:, i - 1:i + 1], osb[:])

        # ---- PE program ----
        # Warm-up: trigger the DVFS ramp; sized to end as the first input
        # slices land (~10us), so real matmuls are never queued behind it.
        warm = const.tile([128, 256], BF16)
        nc.vector.memset(warm[:], 0.0)
        pw = pp_st.tile([128, L], F32, name="psT")
        for _ in range(WARM_N):
            nc.tensor.matmul(pw[:, 0:256], warm[:, 0:128], warm[:],
                             start=True, stop=True)

        u_mm2(0, 2)   # wl m23 is the first wl pair to land (scalar ring)
        u_mm2(0, 0)
        u_mm2(0, 4)
        score_b(0)
        u_mm2(1, 0)
        out_chunk(0, 0, "scalar")
        out_chunk(0, 1, "vector")
        u_mm2(1, 2)
        out_chunk(0, 2, "scalar")
        out_chunk(0, 3, "vector")
        u_mm2(1, 4)
        score_b(1)
        out_chunk(1, 0, "scalar")
        out_chunk(1, 1, "vector")
        out_chunk(1, 2, "scalar")
        out_chunk(1, 3, "vector")
        score_b(2)
        out_chunk(2, 0, "scalar")
        out_chunk(2, 1, "vector")
        out_chunk(2, 2, "scalar")
        out_chunk(2, 3, "vector")
        score_b(3)
        out_chunk(3, 0, "vector")
        out_chunk(3, 1, "scalar")
        out_chunk(3, 2, "vector")
        out_chunk(3, 3, "split")


_CACHE = {}


def _build():
    if "nc" in _CACHE:
        return _CACHE["nc"]
    nc = bacc.Bacc("TRN2", target_bir_lowering=False, debug=False,
                   num_devices=N_CORES)
    linesT_d = nc.dram_tensor("linesT", [BPC, 128, KC * L], BF16,
                              kind="ExternalInput").ap()
    vT01_d = nc.dram_tensor("vT01", [128, KC * 2 * V], BF16,
                            kind="ExternalInput").ap()
    vT23_d = nc.dram_tensor("vT23", [128, KC * 2 * V], BF16,
                            kind="ExternalInput").ap()
    vones_d = nc.dram_tensor("vones", [128, BPC * DE], BF16,
                             kind="ExternalInput").ap()
    maskb_d = nc.dram_tensor("maskb", [V, BPC], F32, kind="ExternalInput").ap()
    wl_d = nc.dram_tensor("wl", [128, KC * D], BF16, kind="ExternalInput").ap()
    out_d = nc.dram_tensor("out", [BPC, L, D], BF16, kind="ExternalOutput").ap()
    with tile.TileContext(nc) as tc:
        _body(tc, out_d, linesT_d, vT01_d, vT23_d, vones_d, maskb_d, wl_d)
    nc.compile()
    _CACHE["nc"] = nc
    return nc


def _in_maps(lines, videos, video_mask, w_q, w_k):
    w_q = np.asarray(w_q, dtype=np.float32)
    w_k = np.asarray(w_k, dtype=np.float32)
    video_mask = np.asarray(video_mask)
    scale = np.float64(D) ** -0.5
    # scores = lines @ (w_q.T @ w_k * scale) @ videos^T; device wants WL[d', d] = W[d, d']
    WL = (scale * (w_k.astype(np.float64).T @ w_q.astype(np.float64))
          ).astype(np.float32)
    mask_bias = np.where(np.asarray(video_mask) == 0,
                         np.float32(-50.0), np.float32(0.0)).astype(np.float32)
    import ml_dtypes
    bf16 = ml_dtypes.bfloat16
    videos = np.asarray(videos, dtype=np.float32)
    lines = np.asarray(lines, dtype=np.float32)
    # vbr layout [v, (b, d)] per core, with a ones column appended (d=768)
    vones = videos.astype(bf16)
    vones = vones.reshape(N_CORES, BPC, V, D).transpose(0, 2, 1, 3)
    ones_col = np.ones((N_CORES, V, BPC, 1), dtype=bf16)
    vones = np.concatenate([vones, ones_col], axis=-1)
    vones = np.ascontiguousarray(vones.reshape(N_CORES, V, BPC * DE))
    # lT layout [b][p=d%128, (c=d//128, l)] per core
    linesT = lines.transpose(0, 2, 1).astype(bf16)          # [B, D, L]
    linesT = linesT.reshape(B, KC, 128, L).transpose(0, 2, 1, 3)
    linesT = np.ascontiguousarray(linesT.reshape(N_CORES, BPC, 128, KC * L))
    # vT pair layout [p=d'%128, (c, bpair, v)] per core
    videosT = videos.transpose(0, 2, 1).astype(bf16)        # [B, D, V]
    videosT = videosT.reshape(N_CORES, BPC, KC, 128, V).transpose(0, 3, 2, 1, 4)
    vT01 = np.ascontiguousarray(
        videosT[:, :, :, 0:2, :].reshape(N_CORES, 128, KC * 2 * V))
    vT23 = np.ascontiguousarray(
        videosT[:, :, :, 2:4, :].reshape(N_CORES, 128, KC * 2 * V))
    # wl layout [p=d'%128, (m, c, s)] with wl[p, m, c, s] = WL[c*128+p, m*128+s]
    WLh = np.ascontiguousarray(
        WL.astype(bf16).reshape(KC, 128, KC, 128)
        .transpose(1, 2, 0, 3).reshape(128, KC * D))
    maps = []
    for c in range(N_CORES):
        sl = slice(c * BPC, (c + 1) * BPC)
        maps.append({
            "linesT": linesT[c],
            "vT01": vT01[c],
            "vT23": vT23[c],
            "vones": vones[c],
            "maskb": np.ascontiguousarray(mask_bias[sl].T),
            "wl": WLh,
        })
    return maps


def kernel(lines, videos, video_mask, w_q, w_k):
    nc = _build()
    maps = _in_maps(lines, videos, video_mask, w_q, w_k)
    res = run_bass_kernel_spmd(nc, maps, list(range(N_CORES)))
    out = np.concatenate([res.results[c]["out"] for c in range(N_CORES)], axis=0)
    return np.ascontiguousarray(out.astype(np.float32))
